# revision 22
# baseline (speedup 1.0000x reference)
"""Fused LyapunovThinkingBlock kernel for 8x TRN2 NeuronCores.

Math (B=32768, D=896): the reference block is
    q,k unused: softmax over a length-1 axis is exactly 1.0 => ctx == v
    v     = phi_x @ Wv^T + b_v
    h_att = v @ Wo^T + b_o
    g1    = silu([h_t, h_att] @ w1^T + b1)
    g2    = g1 @ w2^T + b2
    out   = h_t + LN(g2) * ln_g + ln_b

Weight folding (host, fp64):
    h_att = phi_x @ (Wo Wv)^T + (Wo b_v + b_o)
    [h_t, h_att] @ w1^T = h_t @ W1a^T + h_att @ W1b^T   (w1 = [W1a | W1b])
    => g1 = silu(xcat @ Wcat + c),  xcat = [h_t, phi_x]  (K = 2D = 1792)
       Wcat = [W1a | W1b Wo Wv]^T,  c = b1 + W1b (Wo b_v + b_o)
    w2 column-centering: w2c^T = w2^T - rowwise-mean  (+ b2 centered)
    makes the LN row-mean exactly 0, so LN(g2) = g2c * rsqrt(mean(g2c^2)+eps).

Device (pure data parallel, batch sharded over 8 cores, 4096 rows each):
    All matmul operands bf16 (FWL weight loads hide under matmuls);
    PSUM accumulation fp32.
    stage 1 (feature-major): y1T[d,r] over 14 K-chunks, silu+bias on
        ScalarE from PSUM -> g1T bf16. Block 0 runs k-outer (2 passes,
        4+3 PSUM banks) so the PE chases the streaming weight DMAs.
    stage 2 (row-major): y2[r,d] = g1 @ w2c^T, activation-as-stationary.
    stage 3: var via bn_stats/bn_aggr, rstd = 1/sqrt(var+eps),
        out = y2*rstd + h_t fused on VectorE (scalar_tensor_tensor).
"""

import numpy as np
import ml_dtypes

import concourse.bacc as bacc
import concourse.bass as bass
import concourse.mybir as mybir
import concourse.tile as tile
from concourse.bass_utils import run_bass_kernel_spmd

B, D = 32768, 896
D2 = 2 * D                    # stage-1 contraction (1792)
N_CORES = 8
RPC = B // N_CORES            # rows per core = 4096
P = 128
KC1 = D2 // P                 # 14 stage-1 K-chunks
KC2 = D // P                  # 7 stage-2 K-chunks / stage-1 M-chunks
BLK = 512                     # rows per block
NBLK = RPC // BLK             # 8
NH = 448                      # stage-2 N chunk (2x448 = 896)
LN_EPS = 1e-5

F32 = mybir.dt.float32
BF16 = mybir.dt.bfloat16
F8E4 = mybir.dt.float8e4
BF = ml_dtypes.bfloat16
F8 = ml_dtypes.float8_e4m3

# stage-1 mixed precision: the low-variance 1024 of the 1792 K-dims
# (all phi_x/Bf features + h_t features 0-127) run as fp8 DoubleRow
# (2 contraction rows/cycle); the high-variance rest stays bf16.
KF8C = 8                      # fp8 k-chunks (4 DoubleRow pairs)
KBFC = 6                      # remaining bf16 k-chunks (xcat chunks 1-6)
SX, SW = 32.0, 2048.0         # fp8 operand scales (descaled in the combine)

# test.py can flip these before calling kernel()
TRACE = False
_last_results = None


def _bcast_ap(ap, parts=P):
    return bass.AP(tensor=ap.tensor, offset=ap.offset, ap=[[0, parts], *ap.ap])


def _build(b2_zero: bool, ln_trivial: bool):
    nc = bacc.Bacc(None, target_bir_lowering=False)

    xcatT = nc.dram_tensor("xcatT", [KBFC * P, RPC], BF16, kind="ExternalInput")
    ht_row = nc.dram_tensor("ht_row", [RPC, D], BF16, kind="ExternalInput")
    WcatT_d = nc.dram_tensor("WcatT", [KBFC * P, D], BF16, kind="ExternalInput")
    Wf8T_d = nc.dram_tensor("Wf8T", [KF8C * P, D], F8E4, kind="ExternalInput")
    xf8T = nc.dram_tensor("xf8T", [KF8C * P, RPC], F8E4, kind="ExternalInput")
    w2cT_d = nc.dram_tensor("w2cT", [D, D], BF16, kind="ExternalInput")
    c_d = nc.dram_tensor("c_t", [P, KC2], F32, kind="ExternalInput")
    if not b2_zero:
        b2_d = nc.dram_tensor("b2c", [D], F32, kind="ExternalInput")
    if not ln_trivial:
        lng_d = nc.dram_tensor("ln_g", [D], F32, kind="ExternalInput")
        lnb_d = nc.dram_tensor("ln_b", [D], F32, kind="ExternalInput")
    # output stored bf16 on-device, upcast to fp32 on host
    out_d = nc.dram_tensor("out", [RPC, D], BF16, kind="ExternalOutput")

    xcatT_v = xcatT.rearrange("(kc p) n -> p kc n", p=P)
    WcatT_v = WcatT_d.rearrange("(kc p) n -> p kc n", p=P)
    Wf8T_v = Wf8T_d.rearrange("(kc p) n -> p kc n", p=P)
    xf8T_v = xf8T.rearrange("(kc p) n -> p kc n", p=P)
    w2cT_v = w2cT_d.rearrange("(kc p) n -> p kc n", p=P)

    mult = mybir.AluOpType.mult
    add = mybir.AluOpType.add

    with tile.TileContext(nc) as tc:
        with (
            tc.tile_pool(name="wpool", bufs=1) as wpool,
            tc.tile_pool(name="xpool", bufs=2) as xpool,
            tc.tile_pool(name="gpool", bufs=2) as gpool,
            tc.tile_pool(name="spool", bufs=8) as spool,
            tc.tile_pool(name="hpool", bufs=3) as hpool,
            tc.tile_pool(name="opool", bufs=3) as opool,
            tc.tile_pool(name="ps1", bufs=4, space="PSUM") as ps1p,
            tc.tile_pool(name="ps2", bufs=4, space="PSUM") as ps2p,
        ):
            # ---- PE warmup: tiny matmuls during the head DMA wait flip
            # the HAM clock gate to 8/8 before real matmuls arrive ----
            wm = wpool.tile([P, P], BF16)
            nc.vector.memset(wm[:], 0.0)
            ps_w = ps1p.tile([P, BLK], F32, tag="ps1")
            for _ in range(24):
                nc.tensor.matmul(ps_w[:, 0:P], wm[:], wm[:],
                                 start=True, stop=True)

            # ---- block-0 x first; weights stream per chunk-unit on the
            # scalar queue so block-0's k-outer passes chase them ----
            xf80 = xpool.tile([P, KF8C, BLK], F8E4, tag="xf8", name="xf8_0")
            nc.sync.dma_start(out=xf80[:], in_=xf8T_v[:, :, 0:BLK])
            xbf0 = xpool.tile([P, KBFC, BLK], BF16, tag="xbf", name="xbf_0")
            nc.sync.dma_start(out=xbf0[:], in_=xcatT_v[:, :, 0:BLK])

            Wf8 = wpool.tile([P, KF8C, D], F8E4)
            for j in range(KF8C // 2):
                nc.scalar.dma_start(out=Wf8[:, 2 * j:2 * j + 2],
                                    in_=Wf8T_v[:, 2 * j:2 * j + 2])
            Wc = wpool.tile([P, KBFC, D], BF16)
            for k in range(KBFC):
                nc.scalar.dma_start(out=Wc[:, k], in_=WcatT_v[:, k])
            cT = wpool.tile([P, KC2], F32)
            nc.sync.dma_start(out=cT[:], in_=c_d[:])
            # w2 tile allocated here; its DMA is issued after block-0
            # stage 1 so its data does not compete during the head.
            w2 = wpool.tile([P, KC2, D], BF16)
            eps_t = wpool.tile([P, 1], F32)
            nc.vector.memset(eps_t[:], LN_EPS)
            if not b2_zero:
                b2b = wpool.tile([P, D], F32)
                nc.gpsimd.dma_start(out=b2b[:], in_=_bcast_ap(b2_d[:]))
            if not ln_trivial:
                lngb = wpool.tile([P, D], F32)
                nc.gpsimd.dma_start(out=lngb[:], in_=_bcast_ap(lng_d[:]))
                lnbb = wpool.tile([P, D], F32)
                nc.gpsimd.dma_start(out=lnbb[:], in_=_bcast_ap(lnb_d[:]))

            htr_v = ht_row.rearrange("(t p) d -> p t d", p=P)

            for blk in range(NBLK):
                cs = slice(blk * BLK, (blk + 1) * BLK)
                if blk == 0:
                    xf8, xbf = xf80, xbf0
                else:
                    xf8 = xpool.tile([P, KF8C, BLK], F8E4, tag="xf8",
                                     name=f"xf8_{blk}")
                    nc.sync.dma_start(out=xf8[:], in_=xf8T_v[:, :, cs])
                    xbf = xpool.tile([P, KBFC, BLK], BF16, tag="xbf",
                                     name=f"xbf_{blk}")
                    nc.sync.dma_start(out=xbf[:], in_=xcatT_v[:, :, cs])


                # ---- stage 1: y1T = Wcat-chunks . xcat-chunks ----
                g1 = gpool.tile([P, KC2, BLK], BF16)
                if blk == 0:
                    # k-outer, 2 passes: PE consumes weight chunk-units
                    # (fp8 DoubleRow pairs, then bf16 chunks) as they
                    # stream in; later units arrive during use.
                    NU = KF8C // 2 + KBFC
                    for ms, me in ((0, 4), (4, 7)):
                        ps = [ps1p.tile([P, BLK], F32, tag="ps1",
                                        name=f"psA_{ms}_{m}")
                              for m in range(ms, me)]
                        for u in range(NU):
                            for m in range(ms, me):
                                msl = slice(m * P, (m + 1) * P)
                                if u < KF8C // 2:
                                    nc.tensor.matmul(
                                        ps[m - ms][:],
                                        Wf8[:, 2 * u:2 * u + 2, msl],
                                        xf80[:, 2 * u:2 * u + 2, :],
                                        start=(u == 0), stop=False,
                                        perf_mode=mybir.MatmulPerfMode.DoubleRow)
                                else:
                                    k = u - KF8C // 2
                                    nc.tensor.matmul(
                                        ps[m - ms][:], Wc[:, k, msl],
                                        xbf0[:, k],
                                        start=False, stop=(u == NU - 1))
                        for m in range(ms, me):
                            nc.scalar.activation(
                                g1[:, m], ps[m - ms][:],
                                mybir.ActivationFunctionType.Silu,
                                bias=cT[:, m:m + 1], scale=1.0 / 65536.0)
                    nc.sync.dma_start(out=w2[:], in_=w2cT_v[:])
                else:
                    # all stage-1 products carry the same 2^16 operand
                    # scale (fp8: 32*2048; bf16: 256*256), so DoubleRow
                    # and bf16 matmuls accumulate into ONE PSUM bank and
                    # the silu descales via its activation scale.
                    for m in range(KC2):
                        msl = slice(m * P, (m + 1) * P)
                        ps1 = ps1p.tile([P, BLK], F32, tag="ps1",
                                        name=f"ps1_{blk}_{m}")
                        for j in range(KF8C // 2):
                            nc.tensor.matmul(
                                ps1[:], Wf8[:, 2 * j:2 * j + 2, msl],
                                xf8[:, 2 * j:2 * j + 2, :],
                                start=(j == 0), stop=False,
                                perf_mode=mybir.MatmulPerfMode.DoubleRow)
                        for k in range(KBFC):
                            nc.tensor.matmul(ps1[:], Wc[:, k, msl],
                                             xbf[:, k],
                                             start=False,
                                             stop=(k == KBFC - 1))
                        nc.scalar.activation(g1[:, m], ps1[:],
                                             mybir.ActivationFunctionType.Silu,
                                             bias=cT[:, m:m + 1],
                                             scale=1.0 / 65536.0)

                # ---- stage 2 + 3 per 128-row tile ----
                htr = hpool.tile([P, BLK // P, D], BF16, tag="htr")
                nc.sync.dma_start(
                    out=htr[:],
                    in_=htr_v[:, blk * (BLK // P):(blk + 1) * (BLK // P)])
                for r in range(BLK // P):
                    rows = slice(blk * BLK + r * P, blk * BLK + (r + 1) * P)
                    rs = slice(r * P, (r + 1) * P)
                    ps2a = ps2p.tile([P, NH], F32, tag="ps2")
                    ps2b = ps2p.tile([P, NH], F32, tag="ps2")
                    for k in range(KC2):
                        nc.tensor.matmul(ps2a[:], g1[:, k, rs], w2[:, k, 0:NH],
                                         start=(k == 0), stop=(k == KC2 - 1))
                    for k in range(KC2):
                        nc.tensor.matmul(ps2b[:], g1[:, k, rs], w2[:, k, NH:D],
                                         start=(k == 0), stop=(k == KC2 - 1))

                    if b2_zero:
                        y0, y1 = ps2a[:], ps2b[:]
                    else:
                        yb = opool.tile([P, D], F32, tag="yb")
                        nc.vector.tensor_add(yb[:, 0:NH], ps2a[:], b2b[:, 0:NH])
                        nc.vector.tensor_add(yb[:, NH:D], ps2b[:], b2b[:, NH:D])
                        y0, y1 = yb[:, 0:NH], yb[:, NH:D]

                    # row mean of y is exactly 0 (w2 columns centered on
                    # host), so LN reduces to y * rsqrt(var + eps).
                    stats = spool.tile([P, 2, 6], F32, tag="stats")
                    nc.vector.bn_stats(out=stats[:, 0], in_=y0)
                    nc.vector.bn_stats(out=stats[:, 1], in_=y1)
                    mv = spool.tile([P, 2], F32, tag="mv")
                    nc.vector.bn_aggr(out=mv[:], in_=stats[:])
                    rstd = spool.tile([P, 1], F32, tag="rstd")
                    nc.scalar.activation(rstd[:], mv[:, 1:2],
                                         mybir.ActivationFunctionType.Sqrt,
                                         bias=eps_t[:], scale=1.0)
                    nc.vector.reciprocal(rstd[:], rstd[:])

                    ht_r = htr[:, r]
                    o = opool.tile([P, D], BF16, tag="o")
                    split_store = blk == NBLK - 1
                    if ln_trivial:
                        # o = y*rstd + h_t in one VectorE pass per half
                        nc.vector.scalar_tensor_tensor(
                            o[:, 0:NH], y0, rstd[:], ht_r[:, 0:NH], mult, add)
                        if split_store:
                            nc.sync.dma_start(out=out_d[rows, 0:NH],
                                              in_=o[:, 0:NH])
                        nc.vector.scalar_tensor_tensor(
                            o[:, NH:D], y1, rstd[:], ht_r[:, NH:D], mult, add)
                    else:
                        hb = opool.tile([P, D], F32, tag="hb")
                        nc.vector.tensor_add(hb[:], ht_r[:], lnbb[:])
                        nrm = opool.tile([P, D], F32, tag="nrm")
                        nc.vector.tensor_scalar_mul(nrm[:, 0:NH], y0, rstd[:])
                        nc.vector.tensor_scalar_mul(nrm[:, NH:D], y1, rstd[:])
                        nc.vector.tensor_mul(nrm[:], nrm[:], lngb[:])
                        nc.vector.tensor_add(o[:], nrm[:], hb[:])
                        split_store = False
                    if split_store:
                        nc.sync.dma_start(out=out_d[rows, NH:D], in_=o[:, NH:D])
                    else:
                        nc.sync.dma_start(out=out_d[rows, :], in_=o[:])

    nc.compile()
    return nc


def kernel(h_t, phi_x, in_proj_w, in_proj_b, out_proj_w, out_proj_b,
           w1, b1, w2, b2, ln_g, ln_b):
    global _last_results

    # ---- host-side weight folding (fp64) ----
    Wv = in_proj_w[2 * D:].astype(np.float64)
    bv = in_proj_b[2 * D:].astype(np.float64)
    Wo = out_proj_w.astype(np.float64)
    W1a = w1[:, :D].astype(np.float64)
    W1b = w1[:, D:].astype(np.float64)
    WoWv = Wo @ Wv
    Bf = W1b @ WoWv
    c = b1.astype(np.float64) + W1b @ (Wo @ bv + out_proj_b.astype(np.float64))

    # stage-1 stationary: [K=2D, M=D] = [W1a | Bf]^T
    WcatT = np.ascontiguousarray(W1a.T[P:D] * 256.0).astype(BF)
    Wf8T = np.clip(np.concatenate([Bf.T, W1a.T[0:P]], axis=0) * SW,
                   -240, 240).astype(F8)
    # stage-2 moving: w2^T with output-column means folded out so the LN
    # row-mean is exactly zero.
    w2T = w2.astype(np.float64).T
    w2cT = (w2T - w2T.mean(axis=1, keepdims=True)).astype(BF)
    c_t = np.ascontiguousarray(c.reshape(KC2, P).T).astype(np.float32)

    b2_zero = bool(np.all(b2 == 0))
    ln_trivial = bool(np.all(ln_g == 1) and np.all(ln_b == 0))

    nc = _build(b2_zero, ln_trivial)

    h_t = np.asarray(h_t, dtype=np.float32)
    phi_x = np.asarray(phi_x, dtype=np.float32)

    in_maps = []
    for i in range(N_CORES):
        rows = slice(i * RPC, (i + 1) * RPC)
        htT = np.ascontiguousarray(h_t[rows].T)
        pxT = np.ascontiguousarray(phi_x[rows].T)
        xcatT = np.ascontiguousarray(htT[P:D] * 256.0).astype(BF)
        xf8T_h = np.clip(np.concatenate([pxT, htT[0:P]], axis=0) * SX,
                         -240, 240).astype(F8)
        m = {
            "xcatT": xcatT,
            "xf8T": xf8T_h,
            "ht_row": np.ascontiguousarray(h_t[rows]).astype(BF),
            "WcatT": WcatT,
            "Wf8T": Wf8T,
            "w2cT": np.ascontiguousarray(w2cT),
            "c_t": c_t,
        }
        if not b2_zero:
            b2c = b2.astype(np.float64)
            m["b2c"] = (b2c - b2c.mean()).astype(np.float32)
        if not ln_trivial:
            m["ln_g"] = np.asarray(ln_g, dtype=np.float32)
            m["ln_b"] = np.asarray(ln_b, dtype=np.float32)
        in_maps.append(m)

    res = run_bass_kernel_spmd(nc, in_maps, core_ids=list(range(N_CORES)),
                               trace=TRACE)
    _last_results = res
    return np.concatenate(
        [r["out"].astype(np.float32) for r in res.results], axis=0)


# revision 23
# speedup vs baseline: 1.0044x; 1.0044x over previous
"""Fused LyapunovThinkingBlock kernel for 8x TRN2 NeuronCores.

Math (B=32768, D=896): the reference block is
    q,k unused: softmax over a length-1 axis is exactly 1.0 => ctx == v
    v     = phi_x @ Wv^T + b_v
    h_att = v @ Wo^T + b_o
    g1    = silu([h_t, h_att] @ w1^T + b1)
    g2    = g1 @ w2^T + b2
    out   = h_t + LN(g2) * ln_g + ln_b

Weight folding (host, fp64):
    h_att = phi_x @ (Wo Wv)^T + (Wo b_v + b_o)
    [h_t, h_att] @ w1^T = h_t @ W1a^T + h_att @ W1b^T   (w1 = [W1a | W1b])
    => g1 = silu(xcat @ Wcat + c),  xcat = [h_t, phi_x]  (K = 2D = 1792)
       Wcat = [W1a | W1b Wo Wv]^T,  c = b1 + W1b (Wo b_v + b_o)
    w2 column-centering: w2c^T = w2^T - rowwise-mean  (+ b2 centered)
    makes the LN row-mean exactly 0, so LN(g2) = g2c * rsqrt(mean(g2c^2)+eps).

Device (pure data parallel, batch sharded over 8 cores, 4096 rows each):
    All matmul operands bf16 (FWL weight loads hide under matmuls);
    PSUM accumulation fp32.
    stage 1 (feature-major): y1T[d,r] over 14 K-chunks, silu+bias on
        ScalarE from PSUM -> g1T bf16. Block 0 runs k-outer (2 passes,
        4+3 PSUM banks) so the PE chases the streaming weight DMAs.
    stage 2 (row-major): y2[r,d] = g1 @ w2c^T, activation-as-stationary.
    stage 3: var via bn_stats/bn_aggr, rstd = 1/sqrt(var+eps),
        out = y2*rstd + h_t fused on VectorE (scalar_tensor_tensor).
"""

import numpy as np
import ml_dtypes

import concourse.bacc as bacc
import concourse.bass as bass
import concourse.mybir as mybir
import concourse.tile as tile
from concourse.bass_utils import run_bass_kernel_spmd

B, D = 32768, 896
D2 = 2 * D                    # stage-1 contraction (1792)
N_CORES = 8
RPC = B // N_CORES            # rows per core = 4096
P = 128
KC1 = D2 // P                 # 14 stage-1 K-chunks
KC2 = D // P                  # 7 stage-2 K-chunks / stage-1 M-chunks
BLK = 512                     # rows per block
NBLK = RPC // BLK             # 8
NH = 448                      # stage-2 N chunk (2x448 = 896)
LN_EPS = 1e-5

F32 = mybir.dt.float32
BF16 = mybir.dt.bfloat16
F8E4 = mybir.dt.float8e4
BF = ml_dtypes.bfloat16
F8 = ml_dtypes.float8_e4m3

# stage-1 mixed precision: the low-variance 1024 of the 1792 K-dims
# (all phi_x/Bf features + h_t features 0-127) run as fp8 DoubleRow
# (2 contraction rows/cycle); the high-variance rest stays bf16.
KF8C = 8                      # fp8 k-chunks (4 DoubleRow pairs)
KBFC = 6                      # remaining bf16 k-chunks (xcat chunks 1-6)
SX, SW = 32.0, 2048.0         # fp8 operand scales (descaled in the combine)

# test.py can flip these before calling kernel()
TRACE = False
_last_results = None


def _bcast_ap(ap, parts=P):
    return bass.AP(tensor=ap.tensor, offset=ap.offset, ap=[[0, parts], *ap.ap])


def _build(b2_zero: bool, ln_trivial: bool):
    nc = bacc.Bacc(None, target_bir_lowering=False)

    xcatT = nc.dram_tensor("xcatT", [KBFC * P, RPC], BF16, kind="ExternalInput")
    ht_row = nc.dram_tensor("ht_row", [RPC, D], BF16, kind="ExternalInput")
    WcatT_d = nc.dram_tensor("WcatT", [KBFC * P, D], BF16, kind="ExternalInput")
    Wf8T_d = nc.dram_tensor("Wf8T", [KF8C * P, D], F8E4, kind="ExternalInput")
    xf8T = nc.dram_tensor("xf8T", [KF8C * P, RPC], F8E4, kind="ExternalInput")
    w2cT_d = nc.dram_tensor("w2cT", [D, D], BF16, kind="ExternalInput")
    c_d = nc.dram_tensor("c_t", [P, KC2], F32, kind="ExternalInput")
    if not b2_zero:
        b2_d = nc.dram_tensor("b2c", [D], F32, kind="ExternalInput")
    if not ln_trivial:
        lng_d = nc.dram_tensor("ln_g", [D], F32, kind="ExternalInput")
        lnb_d = nc.dram_tensor("ln_b", [D], F32, kind="ExternalInput")
    # output stored bf16 on-device, upcast to fp32 on host
    out_d = nc.dram_tensor("out", [RPC, D], BF16, kind="ExternalOutput")

    xcatT_v = xcatT.rearrange("(kc p) n -> p kc n", p=P)
    WcatT_v = WcatT_d.rearrange("(kc p) n -> p kc n", p=P)
    Wf8T_v = Wf8T_d.rearrange("(kc p) n -> p kc n", p=P)
    xf8T_v = xf8T.rearrange("(kc p) n -> p kc n", p=P)
    w2cT_v = w2cT_d.rearrange("(kc p) n -> p kc n", p=P)

    mult = mybir.AluOpType.mult
    add = mybir.AluOpType.add

    with tile.TileContext(nc) as tc:
        with (
            tc.tile_pool(name="wpool", bufs=1) as wpool,
            tc.tile_pool(name="xpool", bufs=2) as xpool,
            tc.tile_pool(name="gpool", bufs=2) as gpool,
            tc.tile_pool(name="spool", bufs=8) as spool,
            tc.tile_pool(name="hpool", bufs=3) as hpool,
            tc.tile_pool(name="opool", bufs=3) as opool,
            tc.tile_pool(name="ps1", bufs=4, space="PSUM") as ps1p,
            tc.tile_pool(name="ps2", bufs=4, space="PSUM") as ps2p,
        ):
            # ---- PE warmup: tiny matmuls during the head DMA wait flip
            # the HAM clock gate to 8/8 before real matmuls arrive ----
            wm = wpool.tile([P, P], BF16)
            nc.vector.memset(wm[:], 0.0)
            ps_w = ps1p.tile([P, BLK], F32, tag="ps1")
            for _ in range(24):
                nc.tensor.matmul(ps_w[:, 0:P], wm[:], wm[:],
                                 start=True, stop=True)

            # ---- block-0 x first; weights stream per chunk-unit on the
            # scalar queue so block-0's k-outer passes chase them ----
            xf80 = xpool.tile([P, KF8C, BLK], F8E4, tag="xf8", name="xf8_0")
            nc.sync.dma_start(out=xf80[:, 0:2], in_=xf8T_v[:, 0:2, 0:BLK])
            nc.sync.dma_start(out=xf80[:, 2:KF8C], in_=xf8T_v[:, 2:KF8C, 0:BLK])
            xbf0 = xpool.tile([P, KBFC, BLK], BF16, tag="xbf", name="xbf_0")
            nc.sync.dma_start(out=xbf0[:], in_=xcatT_v[:, :, 0:BLK])

            Wf8 = wpool.tile([P, KF8C, D], F8E4)
            for j in range(KF8C // 2):
                nc.scalar.dma_start(out=Wf8[:, 2 * j:2 * j + 2],
                                    in_=Wf8T_v[:, 2 * j:2 * j + 2])
            Wc = wpool.tile([P, KBFC, D], BF16)
            for k in range(KBFC):
                nc.scalar.dma_start(out=Wc[:, k], in_=WcatT_v[:, k])
            cT = wpool.tile([P, KC2], F32)
            nc.sync.dma_start(out=cT[:], in_=c_d[:])
            # w2 tile allocated here; its DMA is issued after block-0
            # stage 1 so its data does not compete during the head.
            w2 = wpool.tile([P, KC2, D], BF16)
            eps_t = wpool.tile([P, 1], F32)
            nc.vector.memset(eps_t[:], LN_EPS)
            if not b2_zero:
                b2b = wpool.tile([P, D], F32)
                nc.gpsimd.dma_start(out=b2b[:], in_=_bcast_ap(b2_d[:]))
            if not ln_trivial:
                lngb = wpool.tile([P, D], F32)
                nc.gpsimd.dma_start(out=lngb[:], in_=_bcast_ap(lng_d[:]))
                lnbb = wpool.tile([P, D], F32)
                nc.gpsimd.dma_start(out=lnbb[:], in_=_bcast_ap(lnb_d[:]))

            htr_v = ht_row.rearrange("(t p) d -> p t d", p=P)

            for blk in range(NBLK):
                cs = slice(blk * BLK, (blk + 1) * BLK)
                if blk == 0:
                    xf8, xbf = xf80, xbf0
                else:
                    xf8 = xpool.tile([P, KF8C, BLK], F8E4, tag="xf8",
                                     name=f"xf8_{blk}")
                    nc.sync.dma_start(out=xf8[:], in_=xf8T_v[:, :, cs])
                    xbf = xpool.tile([P, KBFC, BLK], BF16, tag="xbf",
                                     name=f"xbf_{blk}")
                    nc.sync.dma_start(out=xbf[:], in_=xcatT_v[:, :, cs])


                # ---- stage 1: y1T = Wcat-chunks . xcat-chunks ----
                g1 = gpool.tile([P, KC2, BLK], BF16)
                if blk == 0:
                    # k-outer, 2 passes: PE consumes weight chunk-units
                    # (fp8 DoubleRow pairs, then bf16 chunks) as they
                    # stream in; later units arrive during use.
                    NU = KF8C // 2 + KBFC
                    for ms, me in ((0, 4), (4, 7)):
                        ps = [ps1p.tile([P, BLK], F32, tag="ps1",
                                        name=f"psA_{ms}_{m}")
                              for m in range(ms, me)]
                        for u in range(NU):
                            for m in range(ms, me):
                                msl = slice(m * P, (m + 1) * P)
                                if u < KF8C // 2:
                                    nc.tensor.matmul(
                                        ps[m - ms][:],
                                        Wf8[:, 2 * u:2 * u + 2, msl],
                                        xf80[:, 2 * u:2 * u + 2, :],
                                        start=(u == 0), stop=False,
                                        perf_mode=mybir.MatmulPerfMode.DoubleRow)
                                else:
                                    k = u - KF8C // 2
                                    nc.tensor.matmul(
                                        ps[m - ms][:], Wc[:, k, msl],
                                        xbf0[:, k],
                                        start=False, stop=(u == NU - 1))
                        for m in range(ms, me):
                            nc.scalar.activation(
                                g1[:, m], ps[m - ms][:],
                                mybir.ActivationFunctionType.Silu,
                                bias=cT[:, m:m + 1], scale=1.0 / 65536.0)
                    nc.sync.dma_start(out=w2[:], in_=w2cT_v[:])
                else:
                    # all stage-1 products carry the same 2^16 operand
                    # scale (fp8: 32*2048; bf16: 256*256), so DoubleRow
                    # and bf16 matmuls accumulate into ONE PSUM bank and
                    # the silu descales via its activation scale.
                    for m in range(KC2):
                        msl = slice(m * P, (m + 1) * P)
                        ps1 = ps1p.tile([P, BLK], F32, tag="ps1",
                                        name=f"ps1_{blk}_{m}")
                        for j in range(KF8C // 2):
                            nc.tensor.matmul(
                                ps1[:], Wf8[:, 2 * j:2 * j + 2, msl],
                                xf8[:, 2 * j:2 * j + 2, :],
                                start=(j == 0), stop=False,
                                perf_mode=mybir.MatmulPerfMode.DoubleRow)
                        for k in range(KBFC):
                            nc.tensor.matmul(ps1[:], Wc[:, k, msl],
                                             xbf[:, k],
                                             start=False,
                                             stop=(k == KBFC - 1))
                        nc.scalar.activation(g1[:, m], ps1[:],
                                             mybir.ActivationFunctionType.Silu,
                                             bias=cT[:, m:m + 1],
                                             scale=1.0 / 65536.0)

                # ---- stage 2 + 3 per 128-row tile ----
                htr = hpool.tile([P, BLK // P, D], BF16, tag="htr")
                nc.sync.dma_start(
                    out=htr[:],
                    in_=htr_v[:, blk * (BLK // P):(blk + 1) * (BLK // P)])
                for r in range(BLK // P):
                    rows = slice(blk * BLK + r * P, blk * BLK + (r + 1) * P)
                    rs = slice(r * P, (r + 1) * P)
                    ps2a = ps2p.tile([P, NH], F32, tag="ps2")
                    ps2b = ps2p.tile([P, NH], F32, tag="ps2")
                    for k in range(KC2):
                        nc.tensor.matmul(ps2a[:], g1[:, k, rs], w2[:, k, 0:NH],
                                         start=(k == 0), stop=(k == KC2 - 1))
                    for k in range(KC2):
                        nc.tensor.matmul(ps2b[:], g1[:, k, rs], w2[:, k, NH:D],
                                         start=(k == 0), stop=(k == KC2 - 1))

                    if b2_zero:
                        y0, y1 = ps2a[:], ps2b[:]
                    else:
                        yb = opool.tile([P, D], F32, tag="yb")
                        nc.vector.tensor_add(yb[:, 0:NH], ps2a[:], b2b[:, 0:NH])
                        nc.vector.tensor_add(yb[:, NH:D], ps2b[:], b2b[:, NH:D])
                        y0, y1 = yb[:, 0:NH], yb[:, NH:D]

                    # row mean of y is exactly 0 (w2 columns centered on
                    # host), so LN reduces to y * rsqrt(var + eps).
                    stats = spool.tile([P, 2, 6], F32, tag="stats")
                    nc.vector.bn_stats(out=stats[:, 0], in_=y0)
                    nc.vector.bn_stats(out=stats[:, 1], in_=y1)
                    mv = spool.tile([P, 2], F32, tag="mv")
                    nc.vector.bn_aggr(out=mv[:], in_=stats[:])
                    rstd = spool.tile([P, 1], F32, tag="rstd")
                    nc.scalar.activation(rstd[:], mv[:, 1:2],
                                         mybir.ActivationFunctionType.Sqrt,
                                         bias=eps_t[:], scale=1.0)
                    nc.vector.reciprocal(rstd[:], rstd[:])

                    ht_r = htr[:, r]
                    o = opool.tile([P, D], BF16, tag="o")
                    split_store = blk == NBLK - 1
                    if ln_trivial:
                        # o = y*rstd + h_t in one VectorE pass per half
                        nc.vector.scalar_tensor_tensor(
                            o[:, 0:NH], y0, rstd[:], ht_r[:, 0:NH], mult, add)
                        if split_store:
                            nc.sync.dma_start(out=out_d[rows, 0:NH],
                                              in_=o[:, 0:NH])
                        nc.vector.scalar_tensor_tensor(
                            o[:, NH:D], y1, rstd[:], ht_r[:, NH:D], mult, add)
                    else:
                        hb = opool.tile([P, D], F32, tag="hb")
                        nc.vector.tensor_add(hb[:], ht_r[:], lnbb[:])
                        nrm = opool.tile([P, D], F32, tag="nrm")
                        nc.vector.tensor_scalar_mul(nrm[:, 0:NH], y0, rstd[:])
                        nc.vector.tensor_scalar_mul(nrm[:, NH:D], y1, rstd[:])
                        nc.vector.tensor_mul(nrm[:], nrm[:], lngb[:])
                        nc.vector.tensor_add(o[:], nrm[:], hb[:])
                        split_store = False
                    if split_store:
                        nc.sync.dma_start(out=out_d[rows, NH:D], in_=o[:, NH:D])
                    else:
                        nc.sync.dma_start(out=out_d[rows, :], in_=o[:])

    nc.compile()
    return nc


def kernel(h_t, phi_x, in_proj_w, in_proj_b, out_proj_w, out_proj_b,
           w1, b1, w2, b2, ln_g, ln_b):
    global _last_results

    # ---- host-side weight folding (fp64) ----
    Wv = in_proj_w[2 * D:].astype(np.float64)
    bv = in_proj_b[2 * D:].astype(np.float64)
    Wo = out_proj_w.astype(np.float64)
    W1a = w1[:, :D].astype(np.float64)
    W1b = w1[:, D:].astype(np.float64)
    WoWv = Wo @ Wv
    Bf = W1b @ WoWv
    c = b1.astype(np.float64) + W1b @ (Wo @ bv + out_proj_b.astype(np.float64))

    # stage-1 stationary: [K=2D, M=D] = [W1a | Bf]^T
    WcatT = np.ascontiguousarray(W1a.T[P:D] * 256.0).astype(BF)
    Wf8T = np.clip(np.concatenate([Bf.T, W1a.T[0:P]], axis=0) * SW,
                   -240, 240).astype(F8)
    # stage-2 moving: w2^T with output-column means folded out so the LN
    # row-mean is exactly zero.
    w2T = w2.astype(np.float64).T
    w2cT = (w2T - w2T.mean(axis=1, keepdims=True)).astype(BF)
    c_t = np.ascontiguousarray(c.reshape(KC2, P).T).astype(np.float32)

    b2_zero = bool(np.all(b2 == 0))
    ln_trivial = bool(np.all(ln_g == 1) and np.all(ln_b == 0))

    nc = _build(b2_zero, ln_trivial)

    h_t = np.asarray(h_t, dtype=np.float32)
    phi_x = np.asarray(phi_x, dtype=np.float32)

    in_maps = []
    for i in range(N_CORES):
        rows = slice(i * RPC, (i + 1) * RPC)
        htT = np.ascontiguousarray(h_t[rows].T)
        pxT = np.ascontiguousarray(phi_x[rows].T)
        xcatT = np.ascontiguousarray(htT[P:D] * 256.0).astype(BF)
        xf8T_h = np.clip(np.concatenate([pxT, htT[0:P]], axis=0) * SX,
                         -240, 240).astype(F8)
        m = {
            "xcatT": xcatT,
            "xf8T": xf8T_h,
            "ht_row": np.ascontiguousarray(h_t[rows]).astype(BF),
            "WcatT": WcatT,
            "Wf8T": Wf8T,
            "w2cT": np.ascontiguousarray(w2cT),
            "c_t": c_t,
        }
        if not b2_zero:
            b2c = b2.astype(np.float64)
            m["b2c"] = (b2c - b2c.mean()).astype(np.float32)
        if not ln_trivial:
            m["ln_g"] = np.asarray(ln_g, dtype=np.float32)
            m["ln_b"] = np.asarray(ln_b, dtype=np.float32)
        in_maps.append(m)

    res = run_bass_kernel_spmd(nc, in_maps, core_ids=list(range(N_CORES)),
                               trace=TRACE)
    _last_results = res
    return np.concatenate(
        [r["out"].astype(np.float32) for r in res.results], axis=0)
